# revision 67
# baseline (speedup 1.0000x reference)
"""Trainium2 Bass kernel for a Mixtral decoder layer (attention + top-2 MoE).

Strategy (8 NeuronCores):
  Launch 1 (attention): 2D shard = (batch b in {0,1}) x (head-group g in {0..3},
    4 heads / 256 feature slice each). q/k/v projections, AV, and the partial
    O-projection are fp8e4m3 DoubleRow matmuls (K=256 per instruction, 0.5
    cycles/row); scores stay bf16 because the softmax exp - not the PE - is
    the binding resource. exp is split across ACT (true exp -> fp8) and DVE
    (Schraudolph: uint8 = A*s + B, bitcast to fp8e4m3), with engine choice
    tied to the score-psum slot so the two exp streams never couple. The
    softmax denominator folds into a scaled ones-column of V; the reciprocal
    broadcast goes through a DRAM roundtrip, pipelined two attends deep.
    Host sums the 4 partials per batch.
  Host: residual add, rmsnorm, gating logits, exact top-2 routing, per-expert
    token gather (expert-parallel dispatch done in numpy - free).
  Launch 2 (MoE FFN): expert-parallel - core e owns expert e's w1/w3/w2
    (fp8 DoubleRow packed) and processes its routed tokens (padded to
    capacity C) densely. Routing weights and all fp8 descales are applied on
    the host during the scatter-add.
  Both launches self-check one token against a host fp32 reference and retry
  once to defend against the rare cold-start execution flake.
"""
import os
import sys

import numpy as np
import ml_dtypes

for _p in ("/root/.axon_site", "/root/.axon_site/_ro/trn_rl_repo", "/opt/trn_rl_repo"):
    if os.path.isdir(_p) and _p not in sys.path:
        sys.path.append(_p)

import concourse.tile as tile
from concourse import bacc, mybir
from concourse.bass_utils import run_bass_kernel_spmd

BF16 = ml_dtypes.bfloat16
AF = mybir.ActivationFunctionType
ALU = mybir.AluOpType
DT = mybir.dt

H = 1024
S = 2048
B = 2
NH = 16
D = 64
E = 8
I = 2048
T = B * S
EPS = 1e-5

NCORES = 8
NGRP = 4              # head groups (cores per batch)
NHPC = NH // NGRP     # 4 heads per core
DS = NHPC * D         # 256-wide feature slice per core
TQC = 4               # tq chunks of 512
NTK = S // 128        # 16 tk tiles
NCI = H // 128        # 8 contraction chunks

C = 1088              # MoE expert token capacity (per-expert max on this data ~1087)

F8 = ml_dtypes.float8_e4m3
S_W1 = 16.0           # fp8 pre-scale for w1/w3 (undone in the Silu input scale)
S_W2 = 32.0           # fp8 pre-scale for w2 (undone via web)
F8MAX = 240.0

S_QK = 8.0            # fp8 pre-scale for wq/wk/wv
S_WO = 8.0            # fp8 pre-scale for wo
S_AO = 64.0           # aoT scale (ones column = S_QK / S_AO)
EXP_SCALE = 0.125 / (S_QK * S_QK)          # exp(raw_psum * EXP_SCALE)
EXP_MULT = float(8 * np.log2(np.e)) * EXP_SCALE   # Schraudolph uint8 mult
EXP_BIAS = 55.55                                  # tuned Schraudolph bias


def _f8(a):
    return np.clip(a, -F8MAX, F8MAX).astype(F8)


def _pack_dr(a):
    """[K, M] -> [ki, kt, ko, m] fp8 with k = kt*256 + ko*128 + ki."""
    K, M = a.shape
    return np.ascontiguousarray(
        _f8(a).reshape(K // 256, 2, 128, M).transpose(2, 0, 1, 3))

_CACHE = {}
LAST_RESULTS = []     # BassKernelResults of the last kernel() call (for test harness)
TRACE = os.environ.get("KERNEL_TRACE", "0") == "1"


def _capacity_chunks(cap):
    out, o = [], 0
    while o < cap:
        ln = min(512, cap - o)
        out.append((o, ln))
        o += ln
    return out


def _build_l1():
    # fp8 DoubleRow attention.  All projections + AV + O-proj run as fp8
    # DoubleRow matmuls (0.5 cyc/row, K=256/instr); scores stay bf16 (the
    # elementwise exp is the binding engine, so DR there buys nothing).
    # exp is split between ACT (true exp -> fp8) and DVE (Schraudolph:
    # uint8 = scores*EXP_MULT + EXP_BIAS, bitcast to fp8e4m3).
    # Scale plumbing: q,k = S_QK*true (bf16), v = S_QK*true (fp8), ones col
    # = S_QK/S_AO so aoT = S_AO*true (fp8), h1p = S_WO*S_AO*true (bf16,
    # host divides).
    nc = bacc.Bacc("TRN2", target_bir_lowering=False, debug=False, num_devices=NCORES)
    KTH = H // 256
    xTp = nc.dram_tensor("xTp", [128, KTH, 2, S], DT.float8e4, kind="ExternalInput")
    wqp = nc.dram_tensor("wqp", [128, KTH, 2, DS], DT.float8e4, kind="ExternalInput")
    wkp = nc.dram_tensor("wkp", [128, KTH, 2, DS], DT.float8e4, kind="ExternalInput")
    wvp = nc.dram_tensor("wvp", [128, KTH, 2, DS], DT.float8e4, kind="ExternalInput")
    wop = nc.dram_tensor("wop", [128, 2, H], DT.float8e4, kind="ExternalInput")
    h1p = nc.dram_tensor("h1p", [S, H], DT.bfloat16, kind="ExternalOutput")

    DRM = mybir.MatmulPerfMode.DoubleRow
    with tile.TileContext(nc) as tc:
        with tc.tile_pool(name="wpool", bufs=1) as wpool, \
             tc.tile_pool(name="qk", bufs=1) as qkpool, \
             tc.tile_pool(name="pt", bufs=6) as ptpool, \
             tc.tile_pool(name="ao", bufs=1) as aopool, \
             tc.tile_pool(name="rc", bufs=6) as rcpool, \
             tc.tile_pool(name="rb", bufs=6) as rbpool, \
             tc.tile_pool(name="avs", bufs=3) as avspool, \
             tc.tile_pool(name="hout", bufs=8) as hpool, \
             tc.tile_pool(name="dram", bufs=4, space="DRAM") as drpool, \
             tc.tile_pool(name="pp", bufs=3, space="PSUM") as pp, \
             tc.tile_pool(name="pav", bufs=1, space="PSUM") as pav:

            # x tiles split (kt x tq-half) so the first projection matmul
            # chain only waits for the first 4 small tiles
            xts = [[wpool.tile([128, 2, 1024], DT.float8e4, name=f"x{kt}_{th}",
                               tag=f"x{kt}_{th}") for th in range(2)]
                   for kt in range(KTH)]
            for kt in range(KTH):
                nc.sync.dma_start(xts[kt][0][:], xTp[:, kt, :, 0:1024])
            wq_sb = wpool.tile([128, KTH, 2, DS], DT.float8e4)
            nc.sync.dma_start(wq_sb[:], wqp[:])
            wk_sb = wpool.tile([128, KTH, 2, DS], DT.float8e4)
            nc.sync.dma_start(wk_sb[:], wkp[:])
            for kt in range(KTH):
                nc.sync.dma_start(xts[kt][1][:], xTp[:, kt, :, 1024:2048])
            wv_sb = wpool.tile([128, KTH, 2, DS], DT.float8e4)
            nc.sync.dma_start(wv_sb[:], wvp[:])
            wo_sb = wpool.tile([128, 2, H], DT.float8e4)
            nc.sync.dma_start(wo_sb[:], wop[:])

            # q/k: [128, S] bf16 per pair, partition = (head-in-pair)*64 + d
            qts = [qkpool.tile([128, S], DT.bfloat16, name=f"q{p}", tag=f"q{p}")
                   for p in range(2)]
            kts = [qkpool.tile([128, S], DT.bfloat16, name=f"k{p}", tag=f"k{p}")
                   for p in range(2)]
            # v staged for DR AV: [ki, tkt, head, 68]; col 64 = ones * 0.125
            vstage = qkpool.tile([128, NTK, NHPC, 68], DT.float8e4)

            evac_flip = [0]
            pp_idx = [0]

            def pp_tile(shape, name):
                # pool-slot-aware allocation: slot = idx % bufs; exp engine
                # assignment keys off the slot so each engine has its own
                # independent psum-slot stream (no cross-engine lockstep)
                t = pp.tile(shape, DT.float32, tag="pp", name=name)
                idx = pp_idx[0]
                pp_idx[0] += 1
                return t, idx

            def exp_engine(idx):
                # slot0 -> ACT, slot1 -> DVE, slot2 -> ACT 6 of 8 rotations
                # (global ACT exp share ~0.58; DVE carries recip/norm)
                slot = idx % 3
                if slot == 0:
                    return "act"
                if slot == 1:
                    return "dve"
                return "act" if (idx // 3) % 8 < 8 else "dve"

            def evac(dst, src, act_share=2):
                # PSUM-evacuation copies: act_share of 4 go to ACT
                evac_flip[0] = (evac_flip[0] + 1) % 4
                if evac_flip[0] < act_share:
                    nc.scalar.copy(dst, src)
                else:
                    nc.vector.tensor_copy(dst, src)

            def sevac(dst, src, idx):
                # evacuations alternate ACT/DVE
                evac(dst, src)

            def make_qk(pair, wsb, dst, th, act_share=2):
                # one projection round: q or k for a head pair, one tq half
                ps, idx = pp_tile([128, 1024], "ps")
                for kt in range(KTH):
                    for i in range(2):
                        nc.tensor.matmul(
                            ps[:, i * 512:(i + 1) * 512],
                            wsb[:, kt, :, pair * 128:(pair + 1) * 128],
                            xts[kt][th][:, :, i * 512:(i + 1) * 512],
                            start=(kt == 0), stop=(kt == KTH - 1),
                            perf_mode=DRM,
                        )
                sevac(dst[:, th * 1024:(th + 1) * 1024], ps[:], idx)

            def make_v_round(tkt):
                pv, idx = pp_tile([128, 256], "pv")
                for kt in range(KTH):
                    nc.tensor.matmul(
                        pv[:],
                        xts[kt][tkt // 8][:, :, (tkt % 8) * 128:
                                          (tkt % 8 + 1) * 128],
                        wv_sb[:, kt],
                        start=(kt == 0), stop=(kt == KTH - 1),
                        perf_mode=DRM,
                    )
                evac(vstage[:, tkt, :, 0:64],
                     pv[:].rearrange("p (h d) -> p h d", d=64))

            aoT = aopool.tile([128, 2, S], DT.float8e4)

            def attend(h, tqh, prev_fin, weave=(), wpj=1, exp_force=None):
                # one tq half (1024 queries) of head h; exp units split
                # between ACT (true exp) and DVE (Schraudolph) by psum slot.
                # The previous attend's finisher is emitted in two stages
                # inside this attend (fin_a: av evac + recip + rb roundtrip
                # at j=0; fin_b: the norm at j=4, by which time the rb DMA
                # has landed, so the norm never blocks the DVE queue head).
                # `weave` items (projection/oproj rounds) are emitted wpj
                # per iteration so they overlap the attend instead of
                # forming serial phases.
                pair, hi = h // 2, h % 2
                qt, kt_ = qts[pair], kts[pair]
                av = pav.tile([65, 1024], DT.float32, tag="pav", name="av")
                weave = list(weave)
                pending = None
                for j in range(8):
                    pt = ptpool.tile([128, 2, 1024], DT.float8e4, tag="pt")
                    for ko in range(2):
                        tkt = 2 * j + ko
                        sc, idx = pp_tile([128, 1024], "sc")
                        for i in range(2):
                            nc.tensor.matmul(
                                sc[:, i * 512:(i + 1) * 512],
                                kt_[hi * 64:(hi + 1) * 64,
                                    tkt * 128:(tkt + 1) * 128],
                                qt[hi * 64:(hi + 1) * 64,
                                   tqh * 1024 + i * 512:tqh * 1024 + (i + 1) * 512],
                                start=True, stop=True,
                            )
                        if (exp_force or exp_engine(idx)) == "dve":
                            nc.vector.tensor_scalar(
                                pt[:, ko, :].bitcast(DT.uint8), sc[:],
                                EXP_MULT, EXP_BIAS, ALU.mult, ALU.add)
                        else:
                            nc.scalar.activation(
                                pt[:, ko, :], sc[:], AF.Exp, scale=EXP_SCALE)
                    if prev_fin is not None:
                        if j == 0:
                            prev_fin[0]()
                        elif j == 4:
                            prev_fin[1]()
                    for _ in range(min(wpj, len(weave))):
                        weave.pop(0)()
                    # AV one pair behind the exp pipeline
                    if pending is not None:
                        ptp, jp = pending
                        for i in range(2):
                            nc.tensor.matmul(
                                av[:, i * 512:(i + 1) * 512],
                                vstage[:, 2 * jp:2 * jp + 2, h, 0:65],
                                ptp[:, :, i * 512:(i + 1) * 512],
                                start=(jp == 0), stop=(jp == 7),
                                perf_mode=DRM,
                            )
                    pending = (pt, j)
                ptp, jp = pending
                for i in range(2):
                    nc.tensor.matmul(
                        av[:, i * 512:(i + 1) * 512],
                        vstage[:, 2 * jp:2 * jp + 2, h, 0:65],
                        ptp[:, :, i * 512:(i + 1) * 512],
                        start=(jp == 0), stop=(jp == 7),
                        perf_mode=DRM,
                    )

                state = {}

                def fin_a():
                    # evacuate av on ACT (frees the single pav slot); recip
                    # off SBUF on DVE; fire the broadcast roundtrip
                    av_sb = avspool.tile([65, 1024], DT.float32, tag="avs",
                                         name="av_sb")
                    nc.scalar.copy(av_sb[:, 0:512], av[:, 0:512])
                    nc.vector.tensor_copy(av_sb[:, 512:1024], av[:, 512:1024])
                    rc = rcpool.tile([1, 1024], DT.float32, tag="rc")
                    nc.vector.reciprocal(rc[0:1, :], av_sb[64:65, :])
                    rd = drpool.tile([1, 1024], DT.float32)
                    nc.sync.dma_start(rd[:], rc[:])
                    rb = rbpool.tile([64, 1024], DT.float32, tag="rb")
                    nc.sync.dma_start(rb[:], rd[:].to_broadcast([64, 1024]))
                    state["av_sb"] = av_sb
                    state["rb"] = rb

                def fin_b():
                    nc.vector.tensor_tensor(
                        aoT[hi * 64:(hi + 1) * 64, pair,
                            tqh * 1024:(tqh + 1) * 1024],
                        state["av_sb"][0:64, :], state["rb"][:], ALU.mult)
                return fin_a, fin_b

            def oproj(tt, split=False):
                # O-projection for query-token tile tt (128 queries); tail
                # calls split the evac across both engines so the psum slot
                # frees twice as fast
                po, idx = pp_tile([128, 1024], "po")
                for jc in range(2):
                    nc.tensor.matmul(
                        po[:, jc * 512:(jc + 1) * 512],
                        aoT[:, :, tt * 128:(tt + 1) * 128],
                        wo_sb[:, :, jc * 512:(jc + 1) * 512],
                        start=True, stop=True,
                        perf_mode=DRM,
                    )
                ht = hpool.tile([128, H], DT.bfloat16, tag="ht")
                if split:
                    nc.scalar.copy(ht[:, 0:512], po[:, 0:512])
                    nc.vector.tensor_copy(ht[:, 512:1024], po[:, 512:1024])
                else:
                    sevac(ht[:], po[:], idx)
                nc.sync.dma_start(h1p[tt * 128:(tt + 1) * 128, :], ht[:])

            # serial startup: projections + v, then clean attends (weaving
            # extra work into attends measurably slows their pipeline)
            make_qk(0, wq_sb, qts[0], 0)
            make_qk(0, wq_sb, qts[0], 1)
            make_qk(0, wk_sb, kts[0], 0)
            make_qk(0, wk_sb, kts[0], 1)
            nc.vector.memset(vstage[:, :, :, 64:65], S_QK / S_AO)
            for t in range(NTK):
                make_v_round(t)
            make_qk(1, wq_sb, qts[1], 0)
            make_qk(1, wq_sb, qts[1], 1)
            make_qk(1, wk_sb, kts[1], 0)
            make_qk(1, wk_sb, kts[1], 1)
            fin = None
            for h in range(4):
                fin = attend(h, 0, fin)
            for h in range(4):
                fin = attend(h, 1, fin)
                if h >= 2:
                    for tt in range((h - 2) * 4, (h - 1) * 4):
                        oproj(tt)
            fin[0]()
            fin[1]()
            for tt in range(8, 16):
                oproj(tt, split=True)

    nc.compile()
    nc.finalize()
    return nc


def _build_l2(cap):
    # fp8 DoubleRow MoE FFN: all matmuls in fp8e4m3 with K=256 per
    # instruction (0.5 cycles/row in the PE).  Host pre-packs operands as
    # [ki, kt, ko, m] with contraction index k = kt*256 + ko*128 + ki and
    # pre-scales w1/w3 by S_W1 (undone in the Silu input scale) and folds
    # 1/(S_W1*S_W2) into web.
    nc = bacc.Bacc("TRN2", target_bir_lowering=False, debug=False, num_devices=NCORES)
    KTH = H // 256            # 4 DoubleRow steps for the H contraction
    KTI = I // 256            # 8 DoubleRow steps for the I contraction
    NIC = I // 128
    zeT = nc.dram_tensor("zeT", [128, KTH, 2, cap], DT.float8e4, kind="ExternalInput")
    w1T = nc.dram_tensor("w1T", [128, KTH, 2, I], DT.float8e4, kind="ExternalInput")
    w3T = nc.dram_tensor("w3T", [128, KTH, 2, I], DT.float8e4, kind="ExternalInput")
    w2T = nc.dram_tensor("w2T", [128, KTI, 2, H], DT.float8e4, kind="ExternalInput")
    yT = nc.dram_tensor("yT", [H, cap], DT.bfloat16, kind="ExternalOutput")

    cch = _capacity_chunks(cap)
    DR = mybir.MatmulPerfMode.DoubleRow
    with tile.TileContext(nc) as tc:
        with tc.tile_pool(name="wpool", bufs=1) as wpool, \
             tc.tile_pool(name="hh", bufs=1) as hhpool, \
             tc.tile_pool(name="hs", bufs=4) as hspool, \
             tc.tile_pool(name="yt", bufs=4) as ytpool, \
             tc.tile_pool(name="pg", bufs=8, space="PSUM") as pg:

            # fine-grained tiles ordered by first use: the first h-matmul
            # chain (ic 0, cap-chunk 0) needs only the 8 tiles loaded first,
            # and DMA stays ahead of the PE from there
            zcs = [[wpool.tile([128, 2, 512], DT.float8e4, name=f"zc{c}_0",
                                tag=f"zc{c}_0"),
                    wpool.tile([128, 2, cap - 512], DT.float8e4, name=f"zc{c}_1",
                               tag=f"zc{c}_1")]
                   for c in range(KTH)]

            def zsl(c, j):
                o, ln = cch[j]
                if j == 0:
                    return zcs[c][0][:]
                return zcs[c][1][:, :, o - 512:o - 512 + ln]
            NQ = 2
            QW = I // NQ
            w1cs = [[wpool.tile([128, 2, QW], DT.float8e4, name=f"w1c{c}_{a}",
                                tag=f"w1c{c}_{a}") for a in range(NQ)]
                    for c in range(KTH)]
            w3cs = [[wpool.tile([128, 2, QW], DT.float8e4, name=f"w3c{c}_{a}",
                                tag=f"w3c{c}_{a}") for a in range(NQ)]
                    for c in range(KTH)]
            for c in range(KTH):
                nc.sync.dma_start(zcs[c][0][:], zeT[:, c, :, 0:cch[0][1]])
                nc.sync.dma_start(w1cs[c][0][:], w1T[:, c, :, 0:QW])
            for c in range(KTH):
                nc.sync.dma_start(zcs[c][1][:], zeT[:, c, :, cch[0][1]:cap])
            for c in range(KTH):
                nc.sync.dma_start(w3cs[c][0][:], w3T[:, c, :, 0:QW])
            for a in range(1, NQ):
                for c in range(KTH):
                    nc.sync.dma_start(w1cs[c][a][:], w1T[:, c, :, a * QW:(a + 1) * QW])
                for c in range(KTH):
                    nc.sync.dma_start(w3cs[c][a][:], w3T[:, c, :, a * QW:(a + 1) * QW])

            # hh packed for the y-phase DoubleRow contraction: i = ic*128+p
            # maps to (kt, ko) = (ic//2, ic%2); two halves so the y-phase can
            # start after the first half
            hhs = [hhpool.tile([128, KTI // 2, 2, cap], DT.float8e4, name=f"hh{a}",
                               tag=f"hh{a}") for a in range(2)]
            w2_holder = []

            for ic in range(NIC):
                wa, wo_ = ic // (NIC // NQ), (ic % (NIC // NQ)) * 128
                hp = [pg.tile([128, 512], DT.float32, tag="pg", name=f"hp{j}")
                      for j in range(len(cch))]
                for kt in range(KTH):
                    for j, (o, ln) in enumerate(cch):
                        nc.tensor.matmul(
                            hp[j][:, 0:ln],
                            w1cs[kt][wa][:, :, wo_:wo_ + 128],
                            zsl(kt, j),
                            start=(kt == 0), stop=(kt == KTH - 1),
                            perf_mode=DR,
                        )
                hs = hspool.tile([128, cap], DT.bfloat16, tag="hs", name="hs")
                for j, (o, ln) in enumerate(cch):
                    nc.scalar.activation(hs[:, o:o + ln], hp[j][:, 0:ln], AF.Silu,
                                         scale=1.0 / S_W1)
                gp = [pg.tile([128, 512], DT.float32, tag="pg", name=f"gp{j}")
                      for j in range(len(cch))]
                for kt in range(KTH):
                    for j, (o, ln) in enumerate(cch):
                        nc.tensor.matmul(
                            gp[j][:, 0:ln],
                            w3cs[kt][wa][:, :, wo_:wo_ + 128],
                            zsl(kt, j),
                            start=(kt == 0), stop=(kt == KTH - 1),
                            perf_mode=DR,
                        )
                for j, (o, ln) in enumerate(cch):
                    nc.vector.tensor_tensor(
                        hhs[ic // (NIC // 2)][:, (ic % (NIC // 2)) // 2, ic % 2,
                                              o:o + ln],
                        gp[j][:, 0:ln], hs[:, o:o + ln], ALU.mult)
                if ic == 0:
                    # emit w2 load after the first h-block for DMA priority
                    w2_sb = wpool.tile([128, KTI, 2, H], DT.float8e4)
                    nc.sync.dma_start(w2_sb[:], w2T[:])
                    w2_holder.append(w2_sb)

            w2_sb = w2_holder[0]
            for hc in range(NCI):
                yt = ytpool.tile([128, cap], DT.bfloat16, tag="yt", name="yt")
                for j, (o, ln) in enumerate(cch):
                    # y-phase yp rotates through the (now idle) 6-slot pg
                    # pool so psum handoff never stalls the PE
                    yp = pg.tile([128, 512], DT.float32, tag="pg", name="yp")
                    for kt in range(KTI):
                        nc.tensor.matmul(
                            yp[:, 0:ln],
                            w2_sb[:, kt, :, hc * 128:(hc + 1) * 128],
                            hhs[kt // (KTI // 2)][:, kt % (KTI // 2), :, o:o + ln],
                            start=(kt == 0), stop=(kt == KTI - 1),
                            perf_mode=DR,
                        )
                    # web (routing weight + fp8 descale) is applied on the
                    # host during the scatter-add, so this is a plain copy,
                    # alternating ACT/DVE
                    if j % 2 == 0:
                        nc.vector.tensor_copy(yt[:, o:o + ln], yp[:, 0:ln])
                    else:
                        nc.scalar.copy(yt[:, o:o + ln], yp[:, 0:ln])
                nc.sync.dma_start(yT[hc * 128:(hc + 1) * 128, :], yt[:])

    nc.compile()
    nc.finalize()
    return nc


def _get(name, builder, *args):
    if name not in _CACHE:
        _CACHE[name] = builder(*args)
    return _CACHE[name]


def _rmsnorm(x, w):
    xf = x.astype(np.float32)
    rms = 1.0 / np.sqrt((xf * xf).mean(axis=-1, keepdims=True) + EPS)
    return (xf * rms) * w.astype(np.float32)


def kernel(x, ln1_w, ln2_w, wq, wk, wv, wo, gate_w, w1, w2, w3):
    global LAST_RESULTS
    LAST_RESULTS = []
    x = np.asarray(x, np.float32)
    wq, wk, wv, wo = (np.asarray(a, np.float32) for a in (wq, wk, wv, wo))
    gate_w = np.asarray(gate_w, np.float32)
    w1, w2, w3 = (np.asarray(a, np.float32) for a in (w1, w2, w3))
    ln1_w = np.asarray(ln1_w, np.float32)
    ln2_w = np.asarray(ln2_w, np.float32)

    xf = x.reshape(T, H)
    z1 = _rmsnorm(xf, ln1_w)
    # ---- launch 1: attention (fp8 DoubleRow) ----
    nc1 = _get("l1", _build_l1)
    in_maps = []
    xps = [_pack_dr(np.ascontiguousarray(z1[b * S:(b + 1) * S].T)) for b in range(B)]
    for c in range(NCORES):
        b, g = divmod(c, NGRP)
        sl = slice(g * DS, (g + 1) * DS)
        in_maps.append({
            "xTp": xps[b],
            "wqp": _pack_dr(wq[sl].T * S_QK),
            "wkp": _pack_dr(wk[sl].T * S_QK),
            "wvp": _pack_dr(wv[sl].T * S_QK),
            "wop": _pack_dr(wo[:, sl].T * S_WO)[:, 0],
        })
    inv = 1.0 / (S_WO * S_AO)

    def _check1(res):
        # sanity-check token 0 of core 0 against a host fp32 reference to
        # catch the rare cold-start corruption; loose tol (fp8 kernel)
        q0 = z1[0] @ wq[:DS].T                        # [DS]
        kk = z1[:S] @ wk[:DS].T                       # [S, DS]
        vv = z1[:S] @ wv[:DS].T
        o = np.empty(DS, np.float32)
        for hh in range(NHPC):
            sl_ = slice(hh * D, (hh + 1) * D)
            sc_ = (kk[:, sl_] @ q0[sl_]) * 0.125
            sc_ -= sc_.max()
            p = np.exp(sc_)
            p /= p.sum()
            o[sl_] = p @ vv[:, sl_]
        want = o @ wo[:, :DS].T
        got = res.results[0]["h1p"][0].astype(np.float32) * inv
        num = float(np.linalg.norm(got - want))
        den = float(np.linalg.norm(want)) + 1e-12
        return num / den < 0.35

    res1 = None
    for _attempt in range(2):
        res1 = run_bass_kernel_spmd(nc1, in_maps, core_ids=list(range(NCORES)),
                                    trace=TRACE)
        if _check1(res1):
            break
    LAST_RESULTS.append(res1)

    h1 = xf.copy()
    for c in range(NCORES):
        b = c // NGRP
        h1[b * S:(b + 1) * S] += res1.results[c]["h1p"].astype(np.float32) * inv

    # ---- host: routing (exact fp32 semantics like the reference) ----
    z = _rmsnorm(h1, ln2_w)
    logits = (z.astype(np.float64) @ gate_w.T.astype(np.float64)).astype(np.float32)
    order = np.argsort(-logits, axis=-1, kind="stable")
    sel = order[:, :2]                               # top-2, ties -> lower index
    vals = np.take_along_axis(logits, sel, axis=-1).astype(np.float32)
    mx = vals.max(axis=-1, keepdims=True)
    ex = np.exp(vals - mx)
    rw = (ex / ex.sum(axis=-1, keepdims=True)).astype(np.float32)

    idx_lists = []
    for e in range(E):
        m = (sel == e)
        tok = np.nonzero(m.any(axis=-1))[0]
        wgt = np.where(m, rw, 0.0).sum(axis=-1)[tok]
        idx_lists.append((tok, wgt.astype(np.float32)))
    maxload = max(len(tok) for tok, _ in idx_lists)
    cap = C
    while cap < maxload:
        cap += 64
    nc2 = _get(f"l2_{cap}", _build_l2, cap)

    # ---- launch 2: expert-parallel FFN (fp8 DoubleRow) ----
    zT = np.ascontiguousarray(z.T)                   # [H, T] fp32
    in_maps2 = []
    for e in range(E):
        tok, wgt = idx_lists[e]
        ze = np.zeros((H, cap), np.float32)
        ze[:, :len(tok)] = zT[:, tok]
        in_maps2.append({
            "zeT": _pack_dr(ze),
            "w1T": _pack_dr(w1[e].T * S_W1),
            "w3T": _pack_dr(w3[e].T * S_W1),
            "w2T": _pack_dr(w2[e].T * S_W2),
        })
    inv2 = 1.0 / (S_W1 * S_W2)

    def _check2(res):
        # sanity-check slot 0 of each non-empty expert vs host fp32
        for e in range(E):
            tok, _ = idx_lists[e]
            if len(tok) == 0:
                continue
            zt = z[tok[0]]
            a = zt @ w1[e].T
            want = ((a / (1.0 + np.exp(-a))) * (zt @ w3[e].T)) @ w2[e].T
            got = res.results[e]["yT"][:, 0].astype(np.float32) * inv2
            num = float(np.linalg.norm(got - want))
            den = float(np.linalg.norm(want)) + 1e-12
            if num / den > 0.35:
                return False
        return True

    res2 = None
    for _attempt in range(2):
        res2 = run_bass_kernel_spmd(nc2, in_maps2, core_ids=list(range(NCORES)),
                                    trace=TRACE)
        if _check2(res2):
            break
    LAST_RESULTS.append(res2)

    out = h1.copy()
    for e in range(E):
        tok, wgt = idx_lists[e]
        out[tok] += (res2.results[e]["yT"][:, :len(tok)].T.astype(np.float32)
                     * (wgt * inv2)[:, None])

    return out.reshape(B, S, H).astype(np.float32)

